# revision 40
# baseline (speedup 1.0000x reference)
"""Trainium2 Bass kernel: GQA attention block (B=1, S=2048, DIM=4096, 32 Q / 8 KV
heads, HD=128, RoPE, causal mask, o_proj), tensor-parallel over 8 NeuronCores.

Sharding (per core c) — collective-free:
  - Q heads 4c..4c+3 (wq rows 512c..512c+512), KV head c (wk/wv rows 128c..).
  - x replicated; each core computes qkv projections + RoPE + causal attention
    for its heads, producing ctx^T [512 local features, 2048 seq] in bf16 SBUF.
  - o_proj is CONTRACTION-sharded: each core multiplies its local ctx^T block
    by its 512 rows of wo^T, producing a PARTIAL output [4096 outcols, 2048]
    (f32 accumulate, bf16 store). The host sums the 8 partial outputs - no
    on-device collective at all.

Pipeline: o_proj(sb-1) is emitted AFTER attn(sb) so the Tile list-scheduler
weaves its (ready) matmuls into the exp-paced stalls of attn(sb) - the PE
queue is strict FIFO per the scheduled program, so filler must be emitted
later-but-ready. ds (softmax denominator) accumulates on DVE with a bf16
pair-sum (two probs tiles added in bf16, then one f32 accumulate), o_proj
PSUM evacuations alternate ACT/DVE, exps on ACT - keeping every elementwise
engine below the PE pace. outT writes round-robin across 2 DMA queues.

Causal mask applied structurally: fully-masked key blocks skipped, diagonal
128-col strips use a triangular bf16 multiply, and score/exp/PV/ds work on
diagonal key tiles is trimmed to the valid q suffix. Softmax runs without
max-subtraction (scores bounded ~|10| here) in f32 PSUM.
"""

import numpy as np
import ml_dtypes

B, S, DIM = 1, 2048, 4096
H, HKV, HD = 32, 8, 128
N_CORES = 8
QH = H // N_CORES            # 4 local q heads
OC = QH * HD                 # 512 local q features
SB = 512                     # seq block
NSB = S // SB                # 4
KT = DIM // 128              # 32 contraction tiles
OJ = DIM // 128              # 32 output column tiles (full width per core)
SCALE = HD ** -0.5
NEG = -1e9

bf16 = ml_dtypes.bfloat16

_CACHE = {}


def _build_nc():
    import contextlib
    import concourse.tile as tile
    from concourse import bacc, mybir

    f32 = mybir.dt.float32
    bft = mybir.dt.bfloat16
    AF = mybir.ActivationFunctionType
    ALU = mybir.AluOpType

    nc = bacc.Bacc("TRN2")

    xt_p = nc.declare_dram_parameter("xt_p", [NSB, KT // 4, 128, 4, SB], bft, isOutput=False)
    wq8a = nc.declare_dram_parameter("wq8a", [8, 128, KT // 8, OC // 2], bft, isOutput=False)
    wq8b = nc.declare_dram_parameter("wq8b", [8, 128, KT // 8, OC // 2], bft, isOutput=False)
    wk_p = nc.declare_dram_parameter("wk_p", [128, KT, HD], bft, isOutput=False)
    wv_p = nc.declare_dram_parameter("wv_p", [128, KT, HD], bft, isOutput=False)
    wo_l = nc.declare_dram_parameter("wo_l", [4, 128, OJ, 128], bft, isOutput=False)
    bqc = nc.declare_dram_parameter("bqc", [128, QH], mybir.dt.float32, isOutput=False)
    bkc = nc.declare_dram_parameter("bkc", [128, 1], mybir.dt.float32, isOutput=False)
    bvc = nc.declare_dram_parameter("bvc", [128, 1], mybir.dt.float32, isOutput=False)
    tqc = nc.declare_dram_parameter("tqc", [128, S], bft, isOutput=False)
    tqs = nc.declare_dram_parameter("tqs", [128, S], bft, isOutput=False)
    tkc = nc.declare_dram_parameter("tkc", [128, S], bft, isOutput=False)
    tks = nc.declare_dram_parameter("tks", [128, S], bft, isOutput=False)
    trim = nc.declare_dram_parameter("trim", [128, 128], bft, isOutput=False)
    outT = nc.declare_dram_parameter("outT", [DIM, S], bft, isOutput=True)

    with tile.TileContext(nc) as tc:
        with contextlib.ExitStack() as ctx:
            consts = ctx.enter_context(tc.tile_pool(name="consts", bufs=1))
            xpool = ctx.enter_context(tc.tile_pool(name="xpool", bufs=11))
            persist = ctx.enter_context(tc.tile_pool(name="persist", bufs=4))
            qpool = ctx.enter_context(tc.tile_pool(name="qpool", bufs=2))
            rtmp = ctx.enter_context(tc.tile_pool(name="rtmp", bufs=2))
            ppool = ctx.enter_context(tc.tile_pool(name="ppool", bufs=6))
            npool = ctx.enter_context(tc.tile_pool(name="npool", bufs=2))
            dsum = ctx.enter_context(tc.tile_pool(name="dsum", bufs=2))
            cpool = ctx.enter_context(tc.tile_pool(name="cpool", bufs=8))
            opool = ctx.enter_context(tc.tile_pool(name="opool", bufs=6))

            dpool = ctx.enter_context(tc.tile_pool(name="dpool", bufs=4, space="DRAM"))
            ps_acc = ctx.enter_context(tc.tile_pool(name="ps_acc", bufs=4, space="PSUM"))
            ps_sc = ctx.enter_context(tc.tile_pool(name="ps_sc", bufs=3, space="PSUM"))
            ps_cx = ctx.enter_context(tc.tile_pool(name="ps_cx", bufs=1, space="PSUM"))

            # PE warmup emitted first: N=128 dummy matmuls at the cold clock
            # cover the initial weight/x DMA latency and warm HAM.
            wtile = consts.tile([128, SB], bft)
            nc.vector.memset(wtile[:], 0.0)
            warm_ps = ps_sc.tile([128, SB], f32, tag="sc_ps", name="sc_ps")
            for i in range(80):
                nc.tensor.matmul(warm_ps[:, 0:128], wtile[:, 0:128],
                                 wtile[:, 0:128], start=(i == 0), stop=(i == 79))

            # weights staged in need-order. sync queue: x + wkv. scalar queue:
            # wq in column-split chunks (pass A only needs cols 0:256).
            KQ = KT // 8
            wqa_ch = [None] * 8
            wqb_ch = [None] * 8
            wk_ch = [None] * 4
            wv_ch = [None] * 4

            def load_wqa_chunk(j):
                if j < 8 and wqa_ch[j] is None:
                    w = consts.tile([128, KQ, OC // 2], bft, tag=f"wqa{j}", name=f"wqa{j}")
                    nc.scalar.dma_start(out=w[:], in_=wq8a[j])
                    wqa_ch[j] = w

            def load_wqb_chunk(j):
                if j < 8 and wqb_ch[j] is None:
                    w = consts.tile([128, KQ, OC // 2], bft, tag=f"wqb{j}", name=f"wqb{j}")
                    nc.scalar.dma_start(out=w[:], in_=wq8b[j])
                    wqb_ch[j] = w

            def load_wkv_chunk(j):
                if j < 4 and wk_ch[j] is None:
                    w = consts.tile([128, 8, HD], bft, tag=f"wk{j}", name=f"wk{j}")
                    nc.scalar.dma_start(out=w[:], in_=wk_p[:, 8 * j:8 * (j + 1), :])
                    wk_ch[j] = w
                    w = consts.tile([128, 8, HD], bft, tag=f"wv{j}", name=f"wv{j}")
                    nc.scalar.dma_start(out=w[:], in_=wv_p[:, 8 * j:8 * (j + 1), :])
                    wv_ch[j] = w

            load_wqa_chunk(0)
            load_wkv_chunk(0)
            load_wqa_chunk(1)

            def wq_at(kt, lo, hi):
                if hi <= OC // 2:
                    return wqa_ch[kt // KQ][:, kt % KQ, lo:hi]
                return wqb_ch[kt // KQ][:, kt % KQ, lo - OC // 2:hi - OC // 2]

            def wk_at(kt):
                return wk_ch[kt // 8][:, kt % 8, :]

            def wv_at(kt):
                return wv_ch[kt // 8][:, kt % 8, :]

            # small biases + tables + triangle + wo on gpsimd queue
            bqc_sb = consts.tile([128, QH], f32)
            nc.gpsimd.dma_start(out=bqc_sb[:], in_=bqc[:])
            bkc_sb = consts.tile([128, 1], f32)
            nc.gpsimd.dma_start(out=bkc_sb[:], in_=bkc[:])
            bvc_sb = consts.tile([128, 1], f32)
            nc.gpsimd.dma_start(out=bvc_sb[:], in_=bvc[:])
            tqc_sb = consts.tile([128, S], bft)
            nc.gpsimd.dma_start(out=tqc_sb[:], in_=tqc[:])
            tqs_sb = consts.tile([128, S], bft)
            nc.gpsimd.dma_start(out=tqs_sb[:], in_=tqs[:])
            tkc_sb = consts.tile([128, S], bft)
            nc.gpsimd.dma_start(out=tkc_sb[:], in_=tkc[:])
            tks_sb = consts.tile([128, S], bft)
            nc.gpsimd.dma_start(out=tks_sb[:], in_=tks[:])
            tri_sb = consts.tile([128, 128], bft)
            nc.gpsimd.dma_start(out=tri_sb[:], in_=trim[:])
            ones_cf = consts.tile([128, 1], f32)
            nc.vector.memset(ones_cf[:], 1.0)

            # wo_l chunks: [128, OJ, 128] per local feature tile fi
            wo_ch = []
            for fi in range(4):
                w = consts.tile([128, OJ, 128], bft, tag=f"wo{fi}", name=f"wo{fi}")
                nc.gpsimd.dma_start(out=w[:], in_=wo_l[fi])
                wo_ch.append(w)

            # persistent per-sb K^T and V tiles
            k_tiles = [None] * NSB   # [128 d, SB s] bf16
            v_tiles = [None] * NSB   # [128 s, 4, 128 d] bf16

            def rope(dst, src, tcos, tsin, s0, swp_eng=None):
                # pass-B ropes route the half-swap through the scalar queue:
                # at the proj->attn boundary the sync queue is busy with
                # just-unblocked x prefetch triggers, which would delay the
                # q3 chain (and, via sem-threshold granularity, attention).
                swp_eng = swp_eng or nc.sync
                cL = tcos[0:64, s0:s0 + SB]
                cH = tcos[64:128, s0:s0 + SB]
                sL = tsin[0:64, s0:s0 + SB]
                sH = tsin[64:128, s0:s0 + SB]
                swp = rtmp.tile([128, SB], bft, tag="ropeswp", name="ropeswp")
                swp_eng.dma_start(out=swp[0:64, :], in_=src[64:128, :])
                swp_eng.dma_start(out=swp[64:128, :], in_=src[0:64, :])
                tA = rtmp.tile([128, SB], bft, tag="ropetA", name="ropetA")
                tB = rtmp.tile([128, SB], bft, tag="ropetB", name="ropetB")
                nc.vector.tensor_tensor(tA[0:64, :], src[0:64, :], cL, ALU.mult)
                nc.vector.tensor_tensor(tA[64:128, :], swp[64:128, :], sH, ALU.mult)
                nc.vector.tensor_tensor(tB[0:64, :], swp[0:64, :], sL, ALU.mult)
                nc.vector.tensor_tensor(tB[64:128, :], src[64:128, :], cH, ALU.mult)
                nc.vector.tensor_tensor(dst[0:64, :], tA[0:64, :], tB[0:64, :], ALU.subtract)
                nc.vector.tensor_tensor(dst[64:128, :], tA[64:128, :], tB[64:128, :], ALU.add)

            oq_rr = [nc.sync, nc.gpsimd]

            def o_proj(sb, ctx_tiles):
                # partial o_proj: contraction over the 512 LOCAL features only.
                # last block's outT drains via HW-DGE queues so the epilogue
                # isn't gated on the slow software-DGE queue.
                qs_out = [nc.sync, nc.scalar] if sb == NSB - 1 else oq_rr
                s0 = sb * SB
                for oj in range(OJ):
                    o_ps = ps_acc.tile([128, SB], f32, tag="acc", name="acc")
                    for fi in range(QH):
                        nc.tensor.matmul(
                            o_ps[:], wo_ch[fi][:, oj, :], ctx_tiles[fi][:],
                            start=(fi == 0), stop=(fi == QH - 1),
                        )
                    ot = opool.tile([128, SB], bft, tag="ot", name="ot")
                    if oj % 2 == 0:
                        nc.scalar.activation(ot[:], o_ps[:], AF.Identity)
                    else:
                        nc.vector.tensor_copy(ot[:], o_ps[:])
                    qs_out[oj % 2].dma_start(
                        out=outT[oj * 128:(oj + 1) * 128, s0:s0 + SB], in_=ot[:]
                    )

            ctx_prev = None
            for sb in range(NSB):
                s0 = sb * SB
                q_sb = [None] * QH

                # ---- projection pass A: q0, q1, k, v ----
                qa_ps = [ps_acc.tile([128, SB], f32, tag="acc", name="acc") for _ in range(2)]
                k_ps = ps_acc.tile([128, SB], f32, tag="acc", name="acc")
                v_ps = ps_acc.tile([128, SB], f32, tag="acc", name="acc")
                xt_chunks = [None] * (KT // 4)
                for c4 in range(KT // 4):
                    xt4 = xpool.tile([128, 4, SB], bft, tag="xt", name="xt")
                    nc.sync.dma_start(out=xt4[:], in_=xt_p[sb, c4])
                    xt_chunks[c4] = xt4
                    if sb == 0:
                        load_wqa_chunk(c4 + 2)
                        load_wkv_chunk(c4 + 1)
                        load_wqb_chunk(7 - c4)
                    for k4 in range(4):
                        kt = c4 * 4 + k4
                        xt = xt4[:, k4, :]
                        st = (kt == 0)
                        sp = (kt == KT - 1)
                        for h in range(2):
                            nc.tensor.matmul(
                                qa_ps[h][:], wq_at(kt, h * 128, (h + 1) * 128), xt,
                                start=st, stop=sp,
                            )
                        nc.tensor.matmul(k_ps[:], wk_at(kt), xt, start=st, stop=sp)
                        nc.tensor.matmul(v_ps[:], wv_at(kt), xt, start=st, stop=sp)

                for h in range(2):
                    qraw = qpool.tile([128, SB], bft, tag="qraw", name="qraw")
                    nc.scalar.activation(qraw[:], qa_ps[h][:], AF.Identity,
                                         bias=bqc_sb[:, h:h + 1])
                    qr = qpool.tile([128, SB], bft, tag="qrope", name="qrope", bufs=8)
                    rope(qr, qraw, tqc_sb, tqs_sb, s0)
                    q_sb[h] = qr
                kraw = qpool.tile([128, SB], bft, tag="kraw", name="kraw")
                nc.scalar.activation(kraw[:], k_ps[:], AF.Identity,
                                     bias=bkc_sb[:, 0:1])
                k_t = persist.tile([128, SB], bft, tag="k_t", name="k_t")
                rope(k_t, kraw, tkc_sb, tks_sb, s0)
                k_tiles[sb] = k_t
                vraw = qpool.tile([128, SB], bft, tag="vraw", name="vraw")
                nc.scalar.activation(vraw[:], v_ps[:], AF.Identity,
                                     bias=bvc_sb[:, 0:1])
                v_t = persist.tile([128, QH, 128], bft, tag="v_t", name="v_t")
                for i in range(QH):
                    nc.scalar.dma_start(
                        out=v_t[:, i, :], in_=vraw[:, i * 128:(i + 1) * 128],
                        transpose=True,
                    )
                v_tiles[sb] = v_t

                # ---- projection pass B: q2 fully, then q3, so q2's
                # evac+swap+rope chain hides under q3's matmuls. All x chunks
                # stay resident in the 12-deep pool - no reloads. ----
                for h in range(2):
                    qb_ps = ps_acc.tile([128, SB], f32, tag="acc", name="acc")
                    order = list(reversed(range(KT // 4))) if h == 0 else list(range(KT // 4))
                    nmm = 0
                    for c4 in order:
                        xt4 = xt_chunks[c4]
                        for k4 in range(4):
                            kt = c4 * 4 + k4
                            nmm += 1
                            nc.tensor.matmul(
                                qb_ps[:], wq_at(kt, (2 + h) * 128, (3 + h) * 128),
                                xt4[:, k4, :],
                                start=(nmm == 1), stop=(nmm == KT),
                            )
                    qraw = qpool.tile([128, SB], bft, tag="qraw", name="qraw")
                    nc.scalar.activation(qraw[:], qb_ps[:], AF.Identity,
                                         bias=bqc_sb[:, 2 + h:3 + h])
                    qr = qpool.tile([128, SB], bft, tag="qrope", name="qrope", bufs=8)
                    rope(qr, qraw, tqc_sb, tqs_sb, s0, swp_eng=nc.scalar)
                    q_sb[2 + h] = qr

                # ---- causal attention for q-block sb, 4 heads ----
                nkt2 = 4 * (sb + 1)
                ctx_tiles = [None] * QH
                for h in range(QH):
                    ctx_ps = ps_cx.tile([128, SB], f32, tag="ctx_ps", name="ctx_ps")

                    ds = dsum.tile([128, SB], f32, tag="ds", name="ds")
                    ds_init = [False]

                    def emit_score(kt2):
                        ksb, ki = divmod(kt2, 4)
                        q0 = ki * 128 if ksb == sb else 0
                        sc_ps = ps_sc.tile([128, SB], f32, tag="sc_ps", name="sc_ps")
                        nc.tensor.matmul(
                            sc_ps[:, q0:SB],
                            k_tiles[ksb][:, ki * 128:(ki + 1) * 128],
                            q_sb[h][:, q0:SB],
                            start=True, stop=True,
                        )
                        probs = ppool.tile([128, SB], bft, tag="probs", name="probs", bufs=9)
                        nc.scalar.activation(probs[:, q0:SB], sc_ps[:, q0:SB], AF.Exp)
                        if ksb == sb:
                            # triangular mask on the 128-wide diagonal strip
                            nc.vector.tensor_tensor(
                                probs[:, q0:q0 + 128], probs[:, q0:q0 + 128],
                                tri_sb[:], ALU.mult
                            )
                        return probs, q0

                    def emit_pv(kt2, probs, q0):
                        ksb, ki = divmod(kt2, 4)
                        st = (kt2 == 0)
                        sp = (kt2 == nkt2 - 1)
                        nc.tensor.matmul(
                            ctx_ps[:, q0:SB], v_tiles[ksb][:, ki, :], probs[:, q0:SB],
                            start=st, stop=sp,
                        )

                    def emit_ds_pair(pa, q0a, pb, q0b):
                        # ds += pa + pb with the pair pre-summed in bf16 (2x DVE
                        # rate); q0a <= q0b by kt2 order. The [q0a, q0b) strip
                        # only has pa's contribution.
                        tmp = dsum.tile([128, SB], bft, tag="dstmp", name="dstmp")
                        nc.vector.tensor_tensor(tmp[:, q0b:SB], pa[:, q0b:SB],
                                                pb[:, q0b:SB], ALU.add)
                        if not ds_init[0]:
                            nc.vector.tensor_copy(ds[:, q0b:SB], tmp[:, q0b:SB])
                            if q0b > q0a:
                                nc.vector.tensor_copy(ds[:, q0a:q0b], pa[:, q0a:q0b])
                            ds_init[0] = True
                        else:
                            nc.vector.tensor_tensor(ds[:, q0b:SB], ds[:, q0b:SB],
                                                    tmp[:, q0b:SB], ALU.add)
                            if q0b > q0a:
                                nc.vector.tensor_tensor(ds[:, q0a:q0b], ds[:, q0a:q0b],
                                                        pa[:, q0a:q0b], ALU.add)

                    fifo = [emit_score(k) for k in range(min(3, nkt2))]
                    pair = []
                    for kt2 in range(nkt2):
                        if kt2 + 3 < nkt2:
                            fifo.append(emit_score(kt2 + 3))
                        probs, q0 = fifo.pop(0)
                        emit_pv(kt2, probs, q0)
                        pair.append((probs, q0))
                        if len(pair) == 2:
                            emit_ds_pair(pair[0][0], pair[0][1], pair[1][0], pair[1][1])
                            pair = []
                    den_ps = ps_sc.tile([1, SB], f32, tag="sc_ps", name="sc_ps")
                    nc.tensor.matmul(den_ps[:], ones_cf[:, 0:1], ds[:], start=True, stop=True)
                    recip = npool.tile([1, SB], f32, tag="recip", name="recip")
                    nc.vector.reciprocal_approx_fast(recip[:], den_ps[:])
                    rb = dpool.tile([1, SB], f32, tag="rb", name="rb")
                    nc.gpsimd.dma_start(out=rb[:], in_=recip[:])
                    bc_s = npool.tile([128, SB], f32, tag="bc_s", name="bc_s")
                    nc.gpsimd.dma_start(out=bc_s[:], in_=rb[:].to_broadcast([128, SB]))
                    ctx_sb = cpool.tile([128, SB], bft, tag="ctx_sb", name="ctx_sb")
                    nc.vector.tensor_tensor(ctx_sb[:], ctx_ps[:], bc_s[:], ALU.mult)
                    ctx_tiles[h] = ctx_sb

                # o_proj of the PREVIOUS block: emitted after attn(sb) so its
                # ready matmuls fill attn(sb)'s exp-paced PE stalls.
                if ctx_prev is not None:
                    o_proj(sb - 1, ctx_prev)
                ctx_prev = ctx_tiles

            o_proj(NSB - 1, ctx_prev)

    nc.finalize()
    return nc


def _get_nc():
    if "nc" not in _CACHE:
        _CACHE["nc"] = _build_nc()
    return _CACHE["nc"]


def _make_in_maps(x, freqs_cos, freqs_sin, wq, bq, wk, bk, wv, bv, wo):
    x2 = np.ascontiguousarray(np.asarray(x).reshape(S, DIM))
    xT = np.ascontiguousarray(x2.T)
    xt_p = np.ascontiguousarray(
        xT.reshape(KT // 4, 4, 128, NSB, SB).transpose(3, 0, 2, 1, 4))
    cos = np.asarray(freqs_cos, dtype=np.float32)
    sin = np.asarray(freqs_sin, dtype=np.float32)
    def dup(t):
        return np.ascontiguousarray(np.concatenate([t, t], axis=0).astype(bf16))
    tqc_np = dup(cos.T * SCALE)
    tqs_np = dup(sin.T * SCALE)
    tkc_np = dup(cos.T)
    tks_np = dup(sin.T)
    qq = np.arange(128)[None, :]
    pp = np.arange(128)[:, None]
    tri_np = np.ascontiguousarray((qq >= pp).astype(bf16))
    wq = np.asarray(wq); wk = np.asarray(wk); wv = np.asarray(wv); wo = np.asarray(wo)
    bq = np.asarray(bq); bk = np.asarray(bk); bv = np.asarray(bv)
    in_maps = []
    for c in range(N_CORES):
        qs = slice(c * OC, (c + 1) * OC)
        ks = slice(c * HD, (c + 1) * HD)
        wqT_c = wq[qs].T.astype(bf16)   # [DIM, OC]
        wkT_c = wk[ks].T.astype(bf16)   # [DIM, HD]
        wq8_full = np.ascontiguousarray(
            wqT_c.reshape(8, KT // 8, 128, OC).transpose(0, 2, 1, 3))
        wvT_c = wv[ks].T.astype(bf16)
        woc = wo[:, qs].astype(bf16)    # [DIM out, OC feat]
        wo_l = np.ascontiguousarray(
            woc.reshape(OJ, 128, QH, 128).transpose(2, 3, 0, 1))

        def tile_wkv(wT):
            return np.ascontiguousarray(
                wT.reshape(KT, 128, wT.shape[1]).transpose(1, 0, 2))

        in_maps.append({
            "xt_p": xt_p,
            "wq8a": np.ascontiguousarray(wq8_full[..., 0:OC // 2]),
            "wq8b": np.ascontiguousarray(wq8_full[..., OC // 2:OC]),
            "wk_p": tile_wkv(wkT_c),
            "wv_p": tile_wkv(wvT_c),
            "wo_l": wo_l,
            "bqc": np.ascontiguousarray(bq[qs].astype(np.float32).reshape(QH, HD).T),
            "bkc": np.ascontiguousarray(bk[ks].astype(np.float32).reshape(1, HD).T),
            "bvc": np.ascontiguousarray(bv[ks].astype(np.float32).reshape(1, HD).T),
            "tqc": tqc_np,
            "tqs": tqs_np,
            "tkc": tkc_np,
            "tks": tks_np,
            "trim": tri_np,
        })
    return in_maps


def _assemble(results):
    acc = np.zeros((DIM, S), dtype=np.float32)
    for r in results:
        acc += np.asarray(r["outT"]).astype(np.float32)
    return np.ascontiguousarray(acc.T).astype(bf16).reshape(B, S, DIM)


def _mask_is_causal(mask):
    m = np.asarray(mask, dtype=np.float32)
    ii = np.arange(S, dtype=np.int64)
    expect = np.where(ii[None, :] <= ii[:, None], np.float32(0.0), np.float32(NEG))
    return m.shape == (S, S) and bool(np.array_equal(m, expect))


def _numpy_fallback(x, freqs_cos, freqs_sin, mask, wq, bq, wk, bk, wv, bv, wo):
    xf = np.asarray(x).astype(np.float32).reshape(S, DIM)
    cos = np.asarray(freqs_cos, dtype=np.float32)
    sin = np.asarray(freqs_sin, dtype=np.float32)

    def tb(t):
        return np.asarray(t).astype(np.float32)

    xq = (xf @ tb(wq).T + tb(bq)).astype(bf16).astype(np.float32).reshape(S, H, HD)
    xk = (xf @ tb(wk).T + tb(bk)).astype(bf16).astype(np.float32).reshape(S, HKV, HD)
    xv = (xf @ tb(wv).T + tb(bv)).astype(bf16).astype(np.float32).reshape(S, HKV, HD)

    def rope_np(t):
        half = HD // 2
        a, b = t[..., :half], t[..., half:]
        c = cos[:, None, :]
        s = sin[:, None, :]
        return np.concatenate([a * c - b * s, a * s + b * c], axis=-1)

    xq = rope_np(xq).astype(bf16).astype(np.float32)
    xk = rope_np(xk).astype(bf16).astype(np.float32)
    key = np.repeat(xk, H // HKV, axis=1)
    val = np.repeat(xv, H // HKV, axis=1)
    scores = np.einsum("qhd,khd->hqk", xq, key) * SCALE
    scores = scores + np.asarray(mask, dtype=np.float32)[None]
    scores -= scores.max(axis=-1, keepdims=True)
    p = np.exp(scores)
    p /= p.sum(axis=-1, keepdims=True)
    ctx = np.einsum("hqk,khd->qhd", p.astype(bf16).astype(np.float32), val)
    ctx = ctx.reshape(S, H * HD).astype(bf16).astype(np.float32)
    out = (ctx @ tb(wo).T).astype(bf16)
    return out.reshape(B, S, DIM)


def kernel(x, freqs_cos, freqs_sin, mask, positions, wq, bq, wk, bk, wv, bv, wo,
           _trace=False, _tmpdir=None):
    from concourse.bass_utils import run_bass_kernel_spmd

    if not _mask_is_causal(mask):
        return _numpy_fallback(x, freqs_cos, freqs_sin, mask, wq, bq, wk, bk, wv, bv, wo)

    in_maps = _make_in_maps(x, freqs_cos, freqs_sin, wq, bq, wk, bk, wv, bv, wo)
    nc = _get_nc()
    res = run_bass_kernel_spmd(
        nc, in_maps, core_ids=list(range(N_CORES)), trace=_trace, tmpdir=_tmpdir
    )
    out = _assemble(res.results)
    if _trace:
        return out, res
    return out
